# revision 1
# baseline (speedup 1.0000x reference)
"""Trainium2 Bass kernel for nn_Embedding2Score (session-graph attention +
vocab-scored readout).

Sharding (8 NeuronCores):
  - phase 1 (attention + segment pooling): data-parallel over sessions —
    core k owns graphs [k*128, (k+1)*128) == nodes [k*6400, (k+1)*6400).
  - AllGather of the per-shard pooled vectors s_g^T (tiny, 32KB/core).
  - phase 2 (z = s_h @ item_emb.T): tensor-parallel over the vocab V —
    core k owns item columns [k*12500, (k+1)*12500) and emits z[:, shard].

All matrices are kept in "transposed" (feature-on-partition) layout on
device so every matmul uses the natural [in,out] weight storage as lhsT
with zero on-device transposes. Segment broadcast (v_n -> nodes) and
segment sum are matmuls against 0/1 selector matrices E2 ([graph, node])
and its transpose — constants for the uniform L=50 layout.

Matmul operands are bf16 (PSUM accumulation stays f32, z is written f32):
fp32/fp32r matmuls self-load weights serially; bf16 gets fast weight
loads + 1 cycle/row and halves the input DMA.

Latency hiding: the collectives firmware pays a fixed ~46us rendezvous
barrier before the first gather completes. Each core therefore computes
its OWN graph block's s_h and z first (purely local), overlapping that
window; the remaining 7 blocks read the gathered s_g via rank-rotated
(partition-id-offset) reads, and the host un-rotates the z row blocks.
"""

from contextlib import ExitStack

import numpy as np

H = 128
B = 1024
L = 50
N = B * L
V = 100000
M = 8            # cores
Bs = B // M      # 128 graphs / core
Ns = N // M      # 6400 nodes / core
Vs = V // M      # 12500 vocab cols / core
NT = Ns // H     # 50 node tiles / core
CH = 512         # phase-1a chunk width (nodes)
ZG = 6250        # phase-2 z output group width (vocab cols)


def _sigmoid(x):
    out = np.empty_like(x)
    np.negative(x, out=out)
    np.exp(out, out=out)
    out += 1.0
    np.reciprocal(out, out=out)
    return out


def _kernel_numpy(session, item, batch, W1, b1, W2, b2, q, bq, W3, b3):
    """General-batch fallback (host only). Handles any sorted batch."""
    nb = int(batch.max()) + 1
    last_idx = np.searchsorted(batch, np.arange(nb), side="right") - 1
    v_n = session[last_idx]
    pre = _sigmoid(v_n[batch] @ W1 + b1 + session @ W2 + b2)
    alpha = pre @ q + bq
    w = alpha * session
    s_g = np.zeros((nb, session.shape[1]), np.float32)
    np.add.at(s_g, batch, w)
    s_h = np.concatenate([v_n, s_g], axis=1) @ W3 + b3
    return (s_h @ item.T).astype(np.float32)


def _build_program(bq_val):
    import concourse.bass as bass
    import concourse.bacc as bacc
    import concourse.tile as tile
    from concourse import mybir

    F32 = mybir.dt.float32
    BF16 = mybir.dt.bfloat16
    SIG = mybir.ActivationFunctionType.Sigmoid
    IDN = mybir.ActivationFunctionType.Identity

    nc = bacc.Bacc("TRN2", target_bir_lowering=False, debug=False,
                   num_devices=M)

    # ---- DRAM I/O (per-core data; identical program on all cores) ----
    d_xT = nc.dram_tensor("xT", [H, Ns], BF16, kind="ExternalInput").ap()
    d_e2 = nc.dram_tensor("e2", [Bs, Ns], BF16, kind="ExternalInput").ap()
    d_xnm = nc.dram_tensor("xnm", [H, NT, H], BF16, kind="ExternalInput").ap()
    d_e2t = nc.dram_tensor("e2t", [H, NT, Bs], BF16,
                           kind="ExternalInput").ap()
    d_vno = nc.dram_tensor("vno", [H, Bs], BF16, kind="ExternalInput").ap()
    # v_n^T with columns rotated so local block j = global shard (rank+j)%M
    d_vnf = nc.dram_tensor("vnf", [H, B], BF16, kind="ExternalInput").ap()
    d_item = nc.dram_tensor("itemT", [H, Vs], BF16, kind="ExternalInput").ap()
    d_w1 = nc.dram_tensor("w1", [H, H], BF16, kind="ExternalInput").ap()
    d_w2 = nc.dram_tensor("w2", [H, H], BF16, kind="ExternalInput").ap()
    d_w3a = nc.dram_tensor("w3a", [H, H], BF16, kind="ExternalInput").ap()
    d_w3b = nc.dram_tensor("w3b", [H, H], BF16, kind="ExternalInput").ap()
    d_q = nc.dram_tensor("qv", [H, 1], BF16, kind="ExternalInput").ap()
    d_bc = nc.dram_tensor("bc", [H, 1], F32, kind="ExternalInput").ap()
    d_b3 = nc.dram_tensor("b3", [H, 1], F32, kind="ExternalInput").ap()
    # z rows are in LOCAL block order; the host maps local block j to
    # global graph block (rank+j)%M when assembling the full output.
    d_z = nc.dram_tensor("z", [B, Vs], F32, kind="ExternalOutput").ap()

    cc_in = nc.dram_tensor("cc_in", [H, Bs], BF16).ap()
    cc_out = nc.dram_tensor("cc_out", [M * H, Bs], BF16,
                            addr_space="Shared").ap()

    with tile.TileContext(nc) as tc, ExitStack() as ctx:
        nc_ = tc.nc

        consts = ctx.enter_context(tc.tile_pool(name="consts", bufs=1))
        small = ctx.enter_context(tc.tile_pool(name="small", bufs=1))
        item_pool = ctx.enter_context(tc.tile_pool(name="itemp", bufs=1))
        work = ctx.enter_context(tc.tile_pool(name="work", bufs=3))
        big1 = ctx.enter_context(tc.tile_pool(name="big1", bufs=1))
        zout = ctx.enter_context(tc.tile_pool(name="zout", bufs=4))
        psum_a = ctx.enter_context(
            tc.tile_pool(name="psum_a", bufs=2, space="PSUM"))
        psum_b = ctx.enter_context(
            tc.tile_pool(name="psum_b", bufs=1, space="PSUM"))
        psum_c = ctx.enter_context(
            tc.tile_pool(name="psum_c", bufs=1, space="PSUM"))
        psum_z = ctx.enter_context(
            tc.tile_pool(name="psum_z", bufs=4, space="PSUM"))

        # constants / weights (scalar-engine HWDGE ring carries all loads)
        w1s = consts.tile([H, H], BF16)
        w2s = consts.tile([H, H], BF16)
        w3as = consts.tile([H, H], BF16)
        w3bs = consts.tile([H, H], BF16)
        qs = consts.tile([H, 1], BF16)
        bcs = consts.tile([H, 1], F32)
        b3s = consts.tile([H, 1], F32)
        vno_sb = consts.tile([H, Bs], BF16)
        vnf_sb = consts.tile([H, B], BF16)
        for sb, dr in ((w1s, d_w1), (w2s, d_w2), (w3as, d_w3a),
                       (w3bs, d_w3b), (qs, d_q), (bcs, d_bc), (b3s, d_b3),
                       (vno_sb, d_vno), (vnf_sb, d_vnf)):
            nc_.scalar.dma_start(out=sb[:], in_=dr[:])

        itemT_sb = item_pool.tile([H, Vs], BF16)

        # misc phase-1 results that outlive their producers
        av_sb = small.tile([H, H], BF16)       # (v_n @ W1), graph-major
        alpha_sb = small.tile([H, NT], F32)    # node-tile columns of alpha
        sg_sb = small.tile([H, Bs], BF16)      # s_g^T local shard
        shT_sb = small.tile([H, B], BF16)      # s_h^T, local block order

        xT_sb = big1.tile([H, Ns], BF16)
        e2_sb = big1.tile([Bs, Ns], BF16)
        xnm_sb = big1.tile([H, NT, H], BF16)
        e2t_sb = big1.tile([H, NT, Bs], BF16)
        half = Ns // 2
        nc_.scalar.dma_start(out=xT_sb[:, :half], in_=d_xT[:, :half])
        nc_.scalar.dma_start(out=e2_sb[:, :half], in_=d_e2[:, :half])
        nc_.scalar.dma_start(out=xT_sb[:, half:], in_=d_xT[:, half:])
        nc_.scalar.dma_start(out=e2_sb[:, half:], in_=d_e2[:, half:])
        nc_.scalar.dma_start(out=xnm_sb[:], in_=d_xnm[:])
        nc_.scalar.dma_start(out=e2t_sb[:], in_=d_e2t[:])
        # phase-2 operand: issued last on the load ring, overlaps phase 1
        nc_.scalar.dma_start(out=itemT_sb[:], in_=d_item[:])

        # Av = v_n_shard @ W1   -> [graph, h_out]
        p_av = psum_a.tile([H, CH], F32, tag="pp", name="p_av")
        nc_.tensor.matmul(p_av[:, :H], lhsT=vno_sb[:], rhs=w1s[:],
                          start=True, stop=True)
        nc_.scalar.copy(out=av_sb[:], in_=p_av[:, :H])

        # phase 1a: S^T = sigmoid(W2^T X^T + Av^T E2 + bc) ; alpha columns
        p_alpha = psum_b.tile([H, NT], F32)
        n_chunks = (Ns + CH - 1) // CH
        for c in range(n_chunks):
            c0 = c * CH
            cw = min(CH, Ns - c0)
            pp = psum_a.tile([H, CH], F32, tag="pp")
            nc_.tensor.matmul(pp[:, :cw], lhsT=w2s[:],
                              rhs=xT_sb[:, c0:c0 + cw],
                              start=True, stop=False)
            nc_.tensor.matmul(pp[:, :cw], lhsT=av_sb[:],
                              rhs=e2_sb[:, c0:c0 + cw],
                              start=False, stop=True)
            s_sb = work.tile([H, CH], BF16, tag="schunk")
            nc_.scalar.activation(s_sb[:, :cw], pp[:, :cw], SIG, bias=bcs[:])
            for s in range(cw // H):
                t = c * (CH // H) + s
                nc_.tensor.matmul(p_alpha[:, t:t + 1],
                                  lhsT=s_sb[:, s * H:(s + 1) * H],
                                  rhs=qs[:], start=True, stop=True)
        # alpha = (S^T)^T q + bq, one column per node tile
        nc_.vector.tensor_scalar_add(alpha_sb[:], p_alpha[:], float(bq_val))

        # phase 1c: s_g^T = sum_t (X_t * alpha_t)^T E2T_t
        p_sg = psum_c.tile([H, Bs], F32)
        for t in range(NT):
            xa = work.tile([H, H], BF16, tag="xa")
            nc_.vector.tensor_scalar_mul(xa[:], xnm_sb[:, t, :],
                                         alpha_sb[:, t:t + 1])
            nc_.tensor.matmul(p_sg[:], lhsT=xa[:], rhs=e2t_sb[:, t, :],
                              start=(t == 0), stop=(t == NT - 1))
        nc_.vector.tensor_copy(out=sg_sb[:], in_=p_sg[:])

        # collective: gather every shard's s_g^T (block r = rank r's s_g)
        nc_.scalar.dma_start(out=cc_in[:], in_=sg_sb[:])
        nc_.gpsimd.collective_compute(
            "AllGather", mybir.AluOpType.bypass,
            replica_groups=[list(range(M))],
            ins=[cc_in.opt()], outs=[cc_out.opt()])

        # s_h^T block j = W3a^T vnf_j + W3b^T sg_{(rank+j)%M} + b3
        def sh_block(j, rhs2):
            p_sh = psum_a.tile([H, CH], F32, tag="pp", name=f"p_sh{j}")
            nc_.tensor.matmul(p_sh[:, :Bs], lhsT=w3as[:],
                              rhs=vnf_sb[:, j * Bs:(j + 1) * Bs],
                              start=True, stop=False)
            nc_.tensor.matmul(p_sh[:, :Bs], lhsT=w3bs[:], rhs=rhs2,
                              start=False, stop=True)
            nc_.scalar.activation(shT_sb[:, j * Bs:(j + 1) * Bs],
                                  p_sh[:, :Bs], IDN, bias=b3s[:])

        n_groups = Vs // ZG            # 3 full groups of 4096
        tail = Vs - n_groups * ZG      # 212
        eng_i = 0

        def z_block(bci):
            nonlocal eng_i
            lhs = shT_sb[:, bci * H:(bci + 1) * H]
            for g in range(n_groups + 1):
                g0 = g * ZG
                gw = ZG if g < n_groups else tail
                if gw == 0:
                    continue
                zt = zout.tile([H, ZG], F32, tag="zt")
                for u in range(0, gw, CH):
                    uw = min(CH, gw - u)
                    zp = psum_z.tile([H, CH], F32, tag="zp")
                    nc_.tensor.matmul(zp[:, :uw], lhsT=lhs,
                                      rhs=itemT_sb[:, g0 + u:g0 + u + uw],
                                      start=True, stop=True)
                    if eng_i % 2 == 0:
                        nc_.vector.tensor_copy(out=zt[:, u:u + uw],
                                               in_=zp[:, :uw])
                    else:
                        nc_.scalar.copy(out=zt[:, u:u + uw], in_=zp[:, :uw])
                    eng_i += 1
                ring = nc_.sync if (bci + g) % 2 == 0 \
                    else nc_.scalar
                ring.dma_start(
                    out=d_z[bci * H:(bci + 1) * H, g0:g0 + gw],
                    in_=zt[:, :gw])

        # own block first: fully local, overlaps the collective barrier
        sh_block(0, sg_sb[:])
        z_block(0)

        # remaining blocks read the gathered s_g at rank-rotated offsets
        rank = nc_.sync.partition_id()
        for j in range(1, M):
            rhs2 = work.tile([H, Bs], BF16, tag="rhs2")
            src0 = ((rank + j) % M) * H
            nc_.sync.dma_start(out=rhs2[:],
                               in_=cc_out[bass.ds(src0, H), :])
            sh_block(j, rhs2[:])
        for bci in range(1, M):
            z_block(bci)

    nc.compile()
    return nc


_CACHE = {}


def _get_program(bq_val):
    key = round(float(bq_val), 10)
    if key not in _CACHE:
        _CACHE[key] = _build_program(bq_val)
    return _CACHE[key]


def kernel(session_embedding, item_emb, batch, num_graphs,
           W1, b1, W2, b2, q, bq, W3, b3):
    import ml_dtypes
    BF = ml_dtypes.bfloat16

    session = np.ascontiguousarray(np.asarray(session_embedding, np.float32))
    item = np.ascontiguousarray(np.asarray(item_emb, np.float32))
    batch = np.asarray(batch)
    W1 = np.asarray(W1, np.float32)
    b1 = np.asarray(b1, np.float32)
    W2 = np.asarray(W2, np.float32)
    b2 = np.asarray(b2, np.float32)
    q = np.asarray(q, np.float32)
    bq = np.asarray(bq, np.float32)
    W3 = np.asarray(W3, np.float32)
    b3 = np.asarray(b3, np.float32)

    uniform = (session.shape == (N, H) and item.shape == (V, H)
               and batch.shape == (N,)
               and int(num_graphs) == B
               and np.array_equal(batch, np.repeat(np.arange(B), L)))
    if not uniform:
        return _kernel_numpy(session, item, batch, W1, b1, W2, b2,
                             q, bq, W3, b3)

    from concourse.bass_utils import run_bass_kernel_spmd

    nc = _get_program(bq[0])

    # ---- host-side shard prep (index bookkeeping + bf16 casts) ----
    last_idx = np.arange(B) * L + (L - 1)
    v_n = session[last_idx]                       # [B, H]
    vnfT = np.ascontiguousarray(v_n.T.astype(BF))  # [H, B]

    gidx = (np.arange(Ns) // L).astype(np.int64)
    E2 = np.zeros((Bs, Ns), BF)
    E2[gidx, np.arange(Ns)] = 1.0
    E2T_t = np.ascontiguousarray(
        E2.T.reshape(NT, H, Bs).transpose(1, 0, 2))  # [H, NT, Bs]

    itemT = np.ascontiguousarray(item.T.astype(BF))  # [H, V]
    sessT = session.T.astype(BF)                     # [H, N]

    bc = np.ascontiguousarray((b1 + b2).reshape(H, 1))
    b3c = np.ascontiguousarray(b3.reshape(H, 1))
    qc = np.ascontiguousarray(q.reshape(H, 1).astype(BF))
    W1h = np.ascontiguousarray(W1.astype(BF))
    W2h = np.ascontiguousarray(W2.astype(BF))
    W3a = np.ascontiguousarray(W3[:H].astype(BF))
    W3b = np.ascontiguousarray(W3[H:].astype(BF))

    in_maps = []
    for k in range(M):
        nsl = slice(k * Ns, (k + 1) * Ns)
        xnm_t = np.ascontiguousarray(
            session[nsl].astype(BF).reshape(NT, H, H)
            .transpose(1, 0, 2))                     # [H, NT, H]
        vnf_rot = np.ascontiguousarray(
            np.concatenate([vnfT[:, k * Bs:], vnfT[:, :k * Bs]], axis=1))
        in_maps.append({
            "xT": np.ascontiguousarray(sessT[:, nsl]),
            "e2": E2,
            "xnm": xnm_t,
            "e2t": E2T_t,
            "vno": np.ascontiguousarray(vnfT[:, k * Bs:(k + 1) * Bs]),
            "vnf": vnf_rot,
            "itemT": np.ascontiguousarray(itemT[:, k * Vs:(k + 1) * Vs]),
            "w1": W1h, "w2": W2h, "w3a": W3a, "w3b": W3b,
            "qv": qc, "bc": bc, "b3": b3c,
        })

    res = run_bass_kernel_spmd(nc, in_maps, list(range(M)))

    # un-rotate: core k's local z row-block j holds graphs ((k+j)%M)*Bs..
    z = np.empty((B, V), np.float32)
    for k in range(M):
        zk = res.results[k]["z"]
        for j in range(M):
            gblk = (k + j) % M
            z[gblk * Bs:(gblk + 1) * Bs, k * Vs:(k + 1) * Vs] = \
                zk[j * Bs:(j + 1) * Bs]
    return z



# revision 3
# speedup vs baseline: 1.3350x; 1.3350x over previous
"""Trainium2 Bass kernel for nn_Embedding2Score (session-graph attention +
vocab-scored readout).

Sharding (8 NeuronCores):
  - phase 1 (attention + segment pooling): data-parallel over sessions —
    core k owns graphs [k*128, (k+1)*128) == nodes [k*6400, (k+1)*6400).
  - AllGather of the per-shard pooled vectors s_g^T (tiny, 32KB/core).
  - phase 2 (z = s_h @ item_emb.T): tensor-parallel over the vocab V —
    core k owns item columns [k*12500, (k+1)*12500) and emits z[:, shard].
    Plus E extra "own-row" tiles per core (rows of core k's own graphs x
    the next E vocab shards) that need NO gathered data — they fill the
    DMA pipe while the collective's firmware latency elapses; the
    corresponding column tiles are skipped by their column-owner cores.

Key layout facts:
  - All matmuls keep features on partitions so the natural [in,out]
    weight storage is lhsT with zero on-device transposes.
  - The v_n -> nodes broadcast is a DVE tensor_copy with a stride-0
    (broadcast) inner AP dim (each graph column repeated L=50 times).
  - alpha is computed as a ROW ([1, N]) via q^T @ S matmuls, broadcast
    to all partitions with gpsimd.partition_broadcast, and the ragged
    segment-sum becomes a single DVE windowed reduce (axis X over
    [H, Bs, L]) — no selector matrices and no second copy of X on HBM.
  - z is written bf16 (host upcasts); halves the dominant HBM write.

Latency hiding: the collectives firmware pays a large fixed rendezvous
cost before the first gather completes. A tiny dummy AllGather is issued
at kernel start to absorb what it can of that cost; each core computes
its OWN graph block's s_h and z (plus E extra own-row vocab tiles) while
waiting; the remaining blocks read the gathered s_g via rank-rotated
reads, and the host un-rotates the z row blocks.
"""

from contextlib import ExitStack

import numpy as np

H = 128
B = 1024
L = 50
N = B * L
V = 100000
M = 8            # cores
Bs = B // M      # 128 graphs / core
Ns = N // M      # 6400 nodes / core
Vs = V // M      # 12500 vocab cols / core
CH = 512         # phase-1a chunk width (nodes)
ZG = 6250        # z output group width (vocab cols); 2 groups per tile
E = 1            # extra own-row vocab tiles per core (gather-free fill)


def _sigmoid(x):
    out = np.empty_like(x)
    np.negative(x, out=out)
    np.exp(out, out=out)
    out += 1.0
    np.reciprocal(out, out=out)
    return out


def _kernel_numpy(session, item, batch, W1, b1, W2, b2, q, bq, W3, b3):
    """General-batch fallback (host only). Handles any sorted batch."""
    nb = int(batch.max()) + 1
    last_idx = np.searchsorted(batch, np.arange(nb), side="right") - 1
    v_n = session[last_idx]
    pre = _sigmoid(v_n[batch] @ W1 + b1 + session @ W2 + b2)
    alpha = pre @ q + bq
    w = alpha * session
    s_g = np.zeros((nb, session.shape[1]), np.float32)
    np.add.at(s_g, batch, w)
    s_h = np.concatenate([v_n, s_g], axis=1) @ W3 + b3
    return (s_h @ item.T).astype(np.float32)


def _build_program(bq_val):
    import concourse.bass as bass
    import concourse.bacc as bacc
    import concourse.tile as tile
    from concourse import mybir

    F32 = mybir.dt.float32
    BF16 = mybir.dt.bfloat16
    SIG = mybir.ActivationFunctionType.Sigmoid
    IDN = mybir.ActivationFunctionType.Identity

    nc = bacc.Bacc("TRN2", target_bir_lowering=False, debug=False,
                   num_devices=M)

    # ---- DRAM I/O (per-core data; identical program on all cores) ----
    d_xT = nc.dram_tensor("xT", [H, Ns], BF16, kind="ExternalInput").ap()
    # v_n^T with columns rotated so local block j = global shard (rank+j)%M
    d_vnf = nc.dram_tensor("vnf", [H, B], BF16, kind="ExternalInput").ap()
    # bf16 weights packed: [W1 | W2 | W3a | W3b | q] along the free dim
    d_wp = nc.dram_tensor("wp", [H, 4 * H + 1], BF16,
                          kind="ExternalInput").ap()
    # f32 biases packed: [b1+b2 | b3]
    d_bp = nc.dram_tensor("bp", [H, 2], F32, kind="ExternalInput").ap()
    # item columns: own shard + E following shards (wrapped)
    d_item = nc.dram_tensor("itemT", [H, (1 + E) * Vs], BF16,
                            kind="ExternalInput").ap()
    # z rows are in LOCAL block order; the host maps local block j to
    # global graph block (rank+j)%M. Blocks j >= M-E are never written
    # (their column-owner skipped them; see d_ze).
    d_z = nc.dram_tensor("z", [B, Vs], BF16, kind="ExternalOutput").ap()
    # extra own-row tiles: rows = own graphs, cols = vocab shards
    # (rank+1..rank+E)%M
    d_ze = nc.dram_tensor("ze", [Bs, E * Vs], BF16,
                          kind="ExternalOutput").ap()

    cc_dum_i = nc.dram_tensor("cc_dum_i", [H, 1], BF16).ap()
    cc_dum_o = nc.dram_tensor("cc_dum_o", [M * H, 1], BF16,
                              addr_space="Shared").ap()
    cc_in = nc.dram_tensor("cc_in", [H, Bs], BF16).ap()
    cc_out = nc.dram_tensor("cc_out", [M * H, Bs], BF16,
                            addr_space="Shared").ap()

    with tile.TileContext(nc) as tc, ExitStack() as ctx:
        nc_ = tc.nc

        consts = ctx.enter_context(tc.tile_pool(name="consts", bufs=1))
        small = ctx.enter_context(tc.tile_pool(name="small", bufs=1))
        item_pool = ctx.enter_context(tc.tile_pool(name="itemp", bufs=1))
        work = ctx.enter_context(tc.tile_pool(name="work", bufs=3))
        big1 = ctx.enter_context(tc.tile_pool(name="big1", bufs=1))
        zout = ctx.enter_context(tc.tile_pool(name="zout", bufs=4))
        psum_a = ctx.enter_context(
            tc.tile_pool(name="psum_a", bufs=2, space="PSUM"))
        psum_q = ctx.enter_context(
            tc.tile_pool(name="psum_q", bufs=2, space="PSUM"))
        psum_z = ctx.enter_context(
            tc.tile_pool(name="psum_z", bufs=4, space="PSUM"))

        # ---- dummy collective: absorb the firmware rendezvous early ----
        dum = small.tile([H, 1], BF16)
        nc_.gpsimd.memset(dum[:], 0.0)
        nc_.sync.dma_start(out=cc_dum_i[:], in_=dum[:])
        nc_.gpsimd.collective_compute(
            "AllGather", mybir.AluOpType.bypass,
            replica_groups=[list(range(M))],
            ins=[cc_dum_i.opt()], outs=[cc_dum_o.opt()])

        # ---- input loads (scalar HWDGE ring, in consumption order) ----
        wp_sb = consts.tile([H, 4 * H + 1], BF16)
        bp_sb = consts.tile([H, 2], F32)
        vnf_sb = consts.tile([H, B], BF16)
        nc_.scalar.dma_start(out=wp_sb[:], in_=d_wp[:])
        nc_.scalar.dma_start(out=bp_sb[:], in_=d_bp[:])
        nc_.scalar.dma_start(out=vnf_sb[:], in_=d_vnf[:])
        w1s = wp_sb[:, 0 * H:1 * H]
        w2s = wp_sb[:, 1 * H:2 * H]
        w3as = wp_sb[:, 2 * H:3 * H]
        w3bs = wp_sb[:, 3 * H:4 * H]
        qs = wp_sb[:, 4 * H:4 * H + 1]
        bcs = bp_sb[:, 0:1]
        b3s = bp_sb[:, 1:2]

        xT_sb = big1.tile([H, Ns], BF16)
        half = Ns // 2
        nc_.scalar.dma_start(out=xT_sb[:, :half], in_=d_xT[:, :half])
        nc_.scalar.dma_start(out=xT_sb[:, half:], in_=d_xT[:, half:])

        itemT_sb = item_pool.tile([H, (1 + E) * Vs], BF16)
        for ic in range(1 + E):
            nc_.scalar.dma_start(
                out=itemT_sb[:, ic * Vs:(ic + 1) * Vs],
                in_=d_item[:, ic * Vs:(ic + 1) * Vs])

        # ---- phase 1 prologue: Av^T + bc, broadcast to nodes ----
        av_sb = small.tile([H, Bs], BF16)       # (v_n W1 + bc)^T, own graphs
        avrep = big1.tile([H, Ns], BF16)        # column g repeated L times
        p_av = psum_a.tile([H, CH], F32, tag="pp", name="p_av")
        nc_.tensor.matmul(p_av[:, :Bs], lhsT=w1s, rhs=vnf_sb[:, :Bs],
                          start=True, stop=True)
        nc_.scalar.activation(av_sb[:], p_av[:, :Bs], IDN, bias=bcs)
        nc_.vector.tensor_copy(
            out=avrep[:].rearrange("h (g l) -> h g l", g=Bs, l=L),
            in_=av_sb[:].unsqueeze(2).broadcast_to([H, Bs, L]))

        # ---- phase 1a: S = sigmoid(W2^T X^T + avrep); alpha row = q^T S
        alpharow = small.tile([1, Ns], BF16)
        n_chunks = (Ns + CH - 1) // CH
        for c in range(n_chunks):
            c0 = c * CH
            cw = min(CH, Ns - c0)
            pp = psum_a.tile([H, CH], F32, tag="pp")
            nc_.tensor.matmul(pp[:, :cw], lhsT=w2s,
                              rhs=xT_sb[:, c0:c0 + cw],
                              start=True, stop=True)
            s_sb = work.tile([H, CH], BF16, tag="schunk")
            nc_.vector.tensor_add(s_sb[:, :cw], pp[:, :cw],
                                  avrep[:, c0:c0 + cw])
            nc_.scalar.activation(s_sb[:, :cw], s_sb[:, :cw], SIG)
            pq = psum_q.tile([1, CH], F32, tag="pq")
            nc_.tensor.matmul(pq[:1, :cw], lhsT=qs, rhs=s_sb[:, :cw],
                              start=True, stop=True)
            nc_.vector.tensor_scalar_add(alpharow[:, c0:c0 + cw],
                                         pq[:1, :cw], float(bq_val))

        # ---- phase 1c: Xw = X^T * alpha; s_g^T = windowed sum over L ----
        alpharep = big1.tile([H, Ns], BF16)
        xw_sb = big1.tile([H, Ns], BF16)
        sgf = small.tile([H, Bs], F32)
        sg_sb = small.tile([H, Bs], BF16)
        hh = Ns // 2
        for s0, s1 in ((0, hh), (hh, Ns)):
            nc_.gpsimd.partition_broadcast(alpharep[:, s0:s1],
                                           alpharow[:, s0:s1])
            nc_.vector.tensor_mul(xw_sb[:, s0:s1], xT_sb[:, s0:s1],
                                  alpharep[:, s0:s1])
            nc_.vector.tensor_reduce(
                out=sgf[:, s0 // L:s1 // L],
                in_=xw_sb[:, s0:s1].rearrange("h (g l) -> h g l", l=L),
                axis=mybir.AxisListType.X, op=mybir.AluOpType.add)
        nc_.vector.tensor_copy(out=sg_sb[:], in_=sgf[:])

        # ---- collective: gather every shard's s_g^T ----
        nc_.scalar.dma_start(out=cc_in[:], in_=sg_sb[:])
        nc_.gpsimd.collective_compute(
            "AllGather", mybir.AluOpType.bypass,
            replica_groups=[list(range(M))],
            ins=[cc_in.opt()], outs=[cc_out.opt()])

        # s_h^T block j = W3a^T vnf_j + W3b^T sg_{(rank+j)%M} + b3
        shT_sb = small.tile([H, B], BF16)

        def sh_block(j, rhs2):
            p_sh = psum_a.tile([H, CH], F32, tag="pp", name=f"p_sh{j}")
            nc_.tensor.matmul(p_sh[:, :Bs], lhsT=w3as,
                              rhs=vnf_sb[:, j * Bs:(j + 1) * Bs],
                              start=True, stop=False)
            nc_.tensor.matmul(p_sh[:, :Bs], lhsT=w3bs, rhs=rhs2,
                              start=False, stop=True)
            nc_.scalar.activation(shT_sb[:, j * Bs:(j + 1) * Bs],
                                  p_sh[:, :Bs], IDN, bias=b3s)

        eng_i = 0

        def z_tile(lhs, isrc, dst, ring_sel):
            """[128 rows] x [Vs cols] from item cols isrc, written to dst."""
            nonlocal eng_i
            for g in range(Vs // ZG):
                g0 = g * ZG
                zt = zout.tile([H, ZG], BF16, tag="zt")
                for u in range(0, ZG, CH):
                    uw = min(CH, ZG - u)
                    zp = psum_z.tile([H, CH], F32, tag="zp")
                    nc_.tensor.matmul(
                        zp[:, :uw], lhsT=lhs,
                        rhs=itemT_sb[:, isrc + g0 + u:isrc + g0 + u + uw],
                        start=True, stop=True)
                    if eng_i % 2 == 0:
                        nc_.vector.tensor_copy(out=zt[:, u:u + uw],
                                               in_=zp[:, :uw])
                    else:
                        nc_.scalar.copy(out=zt[:, u:u + uw], in_=zp[:, :uw])
                    eng_i += 1
                ring = nc_.sync if ring_sel % 2 == 0 else nc_.scalar
                ring.dma_start(out=dst[:, g0:g0 + ZG], in_=zt[:])
                ring_sel += 1

        # own block + extra own-row tiles first: fully local, overlaps
        # the collective latency
        sh_block(0, sg_sb[:])
        z_tile(shT_sb[:, 0:H], 0, d_z[0:H, :], 0)
        for i in range(1, E + 1):
            z_tile(shT_sb[:, 0:H], i * Vs,
                   d_ze[:, (i - 1) * Vs:i * Vs], i)

        # remaining blocks read the gathered s_g at rank-rotated offsets;
        # blocks j >= M-E are owned (as own-row tiles) by other cores.
        rank = nc_.sync.partition_id()
        for j in range(1, M - E):
            rhs2 = work.tile([H, Bs], BF16, tag="rhs2")
            src0 = ((rank + j) % M) * H
            nc_.sync.dma_start(out=rhs2[:],
                               in_=cc_out[bass.ds(src0, H), :])
            sh_block(j, rhs2[:])
        for bci in range(1, M - E):
            z_tile(shT_sb[:, bci * H:(bci + 1) * H], 0,
                   d_z[bci * H:(bci + 1) * H, :], bci)

    nc.compile()
    return nc


_CACHE = {}


def _get_program(bq_val):
    key = round(float(bq_val), 10)
    if key not in _CACHE:
        _CACHE[key] = _build_program(bq_val)
    return _CACHE[key]


def kernel(session_embedding, item_emb, batch, num_graphs,
           W1, b1, W2, b2, q, bq, W3, b3):
    import ml_dtypes
    BF = ml_dtypes.bfloat16

    session = np.ascontiguousarray(np.asarray(session_embedding, np.float32))
    item = np.ascontiguousarray(np.asarray(item_emb, np.float32))
    batch = np.asarray(batch)
    W1 = np.asarray(W1, np.float32)
    b1 = np.asarray(b1, np.float32)
    W2 = np.asarray(W2, np.float32)
    b2 = np.asarray(b2, np.float32)
    q = np.asarray(q, np.float32)
    bq = np.asarray(bq, np.float32)
    W3 = np.asarray(W3, np.float32)
    b3 = np.asarray(b3, np.float32)

    uniform = (session.shape == (N, H) and item.shape == (V, H)
               and batch.shape == (N,)
               and int(num_graphs) == B
               and np.array_equal(batch, np.repeat(np.arange(B), L)))
    if not uniform:
        return _kernel_numpy(session, item, batch, W1, b1, W2, b2,
                             q, bq, W3, b3)

    from concourse.bass_utils import run_bass_kernel_spmd

    nc = _get_program(bq[0])

    # ---- host-side shard prep (index bookkeeping + bf16 casts) ----
    last_idx = np.arange(B) * L + (L - 1)
    v_n = session[last_idx]                       # [B, H]
    vnfT = np.ascontiguousarray(v_n.T.astype(BF))  # [H, B]

    itemT = np.ascontiguousarray(item.T.astype(BF))  # [H, V]
    sessT = session.T.astype(BF)                     # [H, N]

    wp = np.concatenate(
        [W1, W2, W3[:H], W3[H:], q.reshape(H, 1)], axis=1).astype(BF)
    wp = np.ascontiguousarray(wp)
    bp = np.ascontiguousarray(
        np.stack([b1 + b2, b3], axis=1).astype(np.float32))

    in_maps = []
    for k in range(M):
        nsl = slice(k * Ns, (k + 1) * Ns)
        vnf_rot = np.ascontiguousarray(
            np.concatenate([vnfT[:, k * Bs:], vnfT[:, :k * Bs]], axis=1))
        icols = [itemT[:, ((k + i) % M) * Vs:(((k + i) % M) + 1) * Vs]
                 for i in range(1 + E)]
        in_maps.append({
            "xT": np.ascontiguousarray(sessT[:, nsl]),
            "vnf": vnf_rot,
            "itemT": np.ascontiguousarray(np.concatenate(icols, axis=1)),
            "wp": wp, "bp": bp,
        })

    res = run_bass_kernel_spmd(nc, in_maps, list(range(M)))

    # un-rotate: core k's local z row-block j holds graphs ((k+j)%M)*Bs..
    # blocks j >= M-E come instead from core (k+j)%M's d_ze extras.
    z = np.empty((B, V), np.float32)
    for k in range(M):
        zk = res.results[k]["z"]
        zek = res.results[k]["ze"]
        for j in range(M - E):
            gblk = (k + j) % M
            z[gblk * Bs:(gblk + 1) * Bs, k * Vs:(k + 1) * Vs] = \
                zk[j * Bs:(j + 1) * Bs].astype(np.float32)
        for i in range(1, E + 1):
            vblk = (k + i) % M
            z[k * Bs:(k + 1) * Bs, vblk * Vs:(vblk + 1) * Vs] = \
                zek[:, (i - 1) * Vs:i * Vs].astype(np.float32)
    return z
